# revision 8
# baseline (speedup 1.0000x reference)
"""CostGlobalEncoder TRN2 kernel: conv3x3(324->128) + global HW x HW attention
+ proj + FFN, data-parallel over batch N=8 across 8 NeuronCores.

Self-contained: hardcodes shapes N=8, D=128, H=48, W=64 (HW=3072).

v2: fp8 e/vT with DoubleRow A@V + in-loop softmax denominators, conv
interleaved into the attention j-loop, exp(-ln(d)) reciprocal on scalar.
"""
import sys
sys.path.insert(0, '/opt/trn_rl_repo')

import numpy as np
import ml_dtypes

import concourse.bass as bass
import concourse.tile as tile
from concourse import mybir
from concourse.bass_utils import run_bass_kernel_spmd

N, D, H, W = 8, 128, 48, 64
HW = H * W                    # 3072
CIN = 324                     # corr channels
KC = 108                      # conv contraction chunk (324 = 3*108)
NT = 6                        # i-tiles of 512 positions
NP = NT // 2                  # i-tile pairs
TI = 512                      # positions per i-tile
RT = TI // W                  # 8 rows per i-tile
NJ = HW // 128                # 24 j-tiles
NJP = NJ // 2                 # 12 j-tile pairs (fp8 DoubleRow)
SCALE = float(D) ** -0.5
EXP_BIAS = -2.5               # exp shift; cancels in softmax normalization

F32 = mybir.dt.float32
BF16 = mybir.dt.bfloat16
F8 = mybir.dt.float8e4
AF = mybir.ActivationFunctionType
DR = mybir.MatmulPerfMode.DoubleRow


def _split_multi_waits(nc, max_waits=1):
    """walrus setupSyncWait rejects instructions with several sem-waits;
    hoist extras onto preceding same-engine NOPs (engines run in order)."""
    for fn in nc.m.functions:
        for blk in fn.blocks:
            insts = blk.instructions
            i = 0
            while i < len(insts):
                inst = insts[i]
                si = inst.sync_info
                if si is not None and si.on_wait and len(si.on_wait) > max_waits:
                    waits = list(si.on_wait)
                    extra, keep = waits[:-max_waits], waits[-max_waits:]
                    nops = []
                    while extra:
                        chunk, extra = extra[:max_waits], extra[max_waits:]
                        nop = mybir.InstNoOp(
                            name=f"waitsplit-{nc.next_id()}", ins=[], outs=[])
                        nop.engine = inst.engine
                        nop.sync_info = mybir.SyncInfo(on_wait=chunk, on_update=[])
                        nops.append(nop)
                    inst.sync_info = mybir.SyncInfo(
                        on_wait=keep, on_update=list(si.on_update))
                    blk.instructions = insts = insts[:i] + nops + insts[i:]
                    i += len(nops)
                i += 1


def build_nc(with_bias=True):
    nc = bass.Bass()
    corr = nc.declare_dram_parameter("corr", [CIN, HW], BF16, isOutput=False)
    k_in = nc.declare_dram_parameter("k", [D, HW], BF16, isOutput=False)
    vT = nc.declare_dram_parameter("vT", [128, NJ, D], F8, isOutput=False)
    wskT = nc.declare_dram_parameter("wskT", [KC, 27, D], BF16, isOutput=False)
    b_sk = nc.declare_dram_parameter("b_sk", [1, D], BF16, isOutput=False)
    wprojT = nc.declare_dram_parameter("wprojT", [2, D, D], BF16, isOutput=False)
    b_proj = nc.declare_dram_parameter("b_proj", [1, D], BF16, isOutput=False)
    wf1T = nc.declare_dram_parameter("wf1T", [D, D], BF16, isOutput=False)
    b_f1 = nc.declare_dram_parameter("b_f1", [D, 1], F32, isOutput=False)
    wf2T = nc.declare_dram_parameter("wf2T", [D, D], BF16, isOutput=False)
    b_f2 = nc.declare_dram_parameter("b_f2", [1, D], BF16, isOutput=False)
    out = nc.declare_dram_parameter("out", [D, HW], F32, isOutput=True)

    with tile.TileContext(nc) as tc:
        with (
            tc.tile_pool(name="const", bufs=1) as cpool,
            tc.tile_pool(name="work", bufs=2) as wpool,
            tc.tile_pool(name="qpool", bufs=8) as qpool,
            tc.tile_pool(name="xpool", bufs=8) as xpool,
            tc.tile_pool(name="epool", bufs=4) as epool,
            tc.tile_pool(name="ps_s", bufs=3, space="PSUM") as ps_s,
            tc.tile_pool(name="ps_av", bufs=2, space="PSUM") as ps_av,
            tc.tile_pool(name="ps_cm", bufs=1, space="PSUM") as ps_cm,
        ):
            # ---- HAM warm-up first: PE busy from the very start so the
            # clock is at 2.4 GHz when the first conv runs ----
            warm = cpool.tile([128, 128], BF16)
            nc.vector.memset(warm[:], 0.0)
            ps_w = ps_cm.tile([128, 128], F32, name="ps_w", tag="c")
            for _ in range(50):
                nc.tensor.matmul(ps_w[:], warm[:], warm[:],
                                 start=True, stop=True)

            # ---- inputs: corr chunks DMA'd straight into the padded
            # tiles, each split across two queues; wskT on gpsimd ----
            corr_pad = []
            for c in range(3):
                cp = cpool.tile([KC, H + 2, W + 2], BF16, name=f"corr_pad{c}")
                nc.vector.memset(cp[:, 0, :], 0.0)
                nc.vector.memset(cp[:, H + 1, :], 0.0)
                nc.vector.memset(cp[:, 1:H + 1, 0:1], 0.0)
                nc.vector.memset(cp[:, 1:H + 1, W + 1:W + 2], 0.0)
                corr_pad.append(cp)
            wskT_sb = cpool.tile([KC, 27, D], BF16)
            for c in range(3):
                nc.gpsimd.dma_start(wskT_sb[:, c * 9:(c + 1) * 9, :],
                                    wskT[:, c * 9:(c + 1) * 9, :])
            HH = H // 2
            for c in range(3):
                src = corr[c * KC:(c + 1) * KC, :].rearrange(
                    "p (h w) -> p h w", h=H)
                nc.sync.dma_start(corr_pad[c][:, 1:HH + 1, 1:W + 1],
                                  src[:, 0:HH, :])
                nc.scalar.dma_start(corr_pad[c][:, HH + 1:H + 1, 1:W + 1],
                                    src[:, HH:H, :])
            k_sb = cpool.tile([D, HW], BF16)
            nc.sync.dma_start(k_sb[:], k_in[:])
            # vT_sb[p, t, d] = v[d, t*128+p], fp8
            vT_sb = cpool.tile([128, NJ, D], F8)
            nc.scalar.dma_start(vT_sb[:], vT[:])
            b_sk_sb = cpool.tile([1, D], BF16)
            nc.gpsimd.dma_start(b_sk_sb[:], b_sk[:])
            wprojT_sb = cpool.tile([D, 2, D], BF16)
            nc.gpsimd.dma_start(wprojT_sb[:], wprojT.rearrange("c p d -> p c d"))
            wf1T_sb = cpool.tile([D, D], BF16)
            nc.gpsimd.dma_start(wf1T_sb[:], wf1T[:])
            wf2T_sb = cpool.tile([D, D], BF16)
            nc.gpsimd.dma_start(wf2T_sb[:], wf2T[:])
            b_proj_sb = cpool.tile([1, D], BF16)
            nc.gpsimd.dma_start(b_proj_sb[:], b_proj[:])
            b_f1_sb = cpool.tile([D, 1], F32)
            nc.gpsimd.dma_start(b_f1_sb[:], b_f1[:])
            b_f2_sb = cpool.tile([1, D], BF16)
            nc.gpsimd.dma_start(b_f2_sb[:], b_f2[:])
            ones_row = cpool.tile([1, TI], BF16)
            nc.gpsimd.memset(ones_row[:], 1.0)
            ones1 = cpool.tile([1, 128], BF16)
            nc.gpsimd.memset(ones1[:], 1.0)
            # fp8 DoubleRow lhsT: k-pair stride must be 16B-aligned
            ones8 = cpool.tile([128, 2, 16], F8)
            nc.gpsimd.memset(ones8[:], 1.0)
            ebias = cpool.tile([128, 1], F32)
            nc.gpsimd.memset(ebias[:], EXP_BIAS)

            def conv_mm0():
                """conv for i-tiles (0, 1) pre-loop, in the av-pool slots.
                c-outer so matmuls start as soon as chunk 0 lands."""
                ps_cs = [ps_av.tile([D, TI], F32, name=f"ps_c0{ii}",
                                    tag="av") for ii in range(2)]
                for c in range(3):
                    for ii in range(2):
                        for t in range(9):
                            dy, dx = t // 3, t % 3
                            y0 = ii * RT
                            nc.tensor.matmul(
                                ps_cs[ii][:],
                                wskT_sb[:, c * 9 + t, :],
                                corr_pad[c][:, y0 + dy:y0 + dy + RT,
                                            dx:dx + W],
                                start=(c == 0 and t == 0),
                                stop=(c == 2 and t == 8 and not with_bias))
                qs, rs = [], []
                for ii in range(2):
                    if with_bias:
                        nc.tensor.matmul(ps_cs[ii][:], b_sk_sb[:],
                                         ones_row[:], start=False, stop=True)
                    q = qpool.tile([D, TI], BF16, name="q")
                    nc.vector.tensor_copy(q[:], ps_cs[ii][:])
                    qs.append(q)
                    resid = qpool.tile([D, TI], F32, name="resid")
                    nc.vector.tensor_copy(resid[:], ps_cs[ii][:])
                    rs.append(resid)
                return qs, rs

            def conv_work(pnext, qn, rn):
                """Closures computing pair pnext's conv, one i-tile at a
                time in the 1-buf "c" PSUM slot; results land in qn/rn."""
                work = []
                st = {}

                def mk_mm(ii, c, t):
                    def f():
                        if st.get('ii') != ii:
                            st['ps'] = ps_cm.tile([D, TI], F32,
                                                  name="ps_c", tag="c")
                            st['ii'] = ii
                        dy, dx = t // 3, t % 3
                        y0 = (2 * pnext + ii) * RT
                        nc.tensor.matmul(
                            st['ps'][:], wskT_sb[:, c * 9 + t, :],
                            corr_pad[c][:, y0 + dy:y0 + dy + RT, dx:dx + W],
                            start=(c == 0 and t == 0),
                            stop=(c == 2 and t == 8 and not with_bias))
                    return f

                def mk_bias(ii):
                    def f():
                        nc.tensor.matmul(st['ps'][:], b_sk_sb[:], ones_row[:],
                                         start=False, stop=True)
                    return f

                def mk_evac(ii):
                    def f():
                        q = qpool.tile([D, TI], BF16, name="q")
                        nc.vector.tensor_copy(q[:], st['ps'][:])
                        qn[ii] = q
                        resid = qpool.tile([D, TI], F32, name="resid")
                        nc.vector.tensor_copy(resid[:], st['ps'][:])
                        rn[ii] = resid
                    return f

                for ii in range(2):
                    for c in range(3):
                        for t in range(9):
                            work.append(mk_mm(ii, c, t))
                    if with_bias:
                        work.append(mk_bias(ii))
                    work.append(mk_evac(ii))
                return work

            def s_pair(j, qs):
                ts = []
                for ii in range(2):
                    t = ps_s.tile([128, TI], F32, name="ps_sj", tag="s")
                    nc.tensor.matmul(t[:],
                                     k_sb[:, j * 128:(j + 1) * 128],
                                     qs[ii][:], start=True, stop=True)
                    ts.append(t)
                return ts

            def normalize(ii, ps_m, ps_aa):
                """1/denominator = exp(-ln(d)) on the scalar engine (idle
                post-loop; ln+exp share an ACT table), broadcast via PE."""
                ln_row = wpool.tile([1, TI], F32, name="ln_row")
                nc.scalar.activation(ln_row[:], ps_m[:], AF.Ln)
                rrow = wpool.tile([1, TI], BF16, name="rrow")
                nc.scalar.activation(rrow[:], ln_row[:], AF.Exp, scale=-1.0)
                ps_b = ps_s.tile([128, TI], F32, name="ps_b", tag="s")
                nc.tensor.matmul(ps_b[:], ones1[:], rrow[:],
                                 start=True, stop=True)
                rb = wpool.tile([128, TI], BF16, name="rb")
                nc.scalar.copy(rb[:], ps_b[:])
                attn = wpool.tile([D, TI], BF16, name="attn")
                nc.vector.tensor_mul(attn[:], ps_aa[:], rb[:])
                return attn

            def proj(attn, resid, q):
                """1x1 proj on concat([attn, resid]) + bias + resid."""
                ps_p = ps_av.tile([D, TI], F32, name="ps_p", tag="av")
                nc.tensor.matmul(ps_p[:], wprojT_sb[:, 0, :], attn[:],
                                 start=True, stop=False)
                nc.tensor.matmul(ps_p[:], wprojT_sb[:, 1, :], q[:],
                                 start=False, stop=not with_bias)
                if with_bias:
                    nc.tensor.matmul(ps_p[:], b_proj_sb[:], ones_row[:],
                                     start=False, stop=True)
                x = xpool.tile([D, TI], F32, name="x")
                nc.vector.tensor_add(x[:], ps_p[:], resid[:])
                x_bf = xpool.tile([D, TI], BF16, name="x_bf")
                nc.vector.tensor_copy(x_bf[:], x[:])
                return x, x_bf

            def ffn(xv, i):
                x, x_bf = xv
                ps_f1 = ps_s.tile([D, TI], F32, name="ps_f1", tag="s")
                nc.tensor.matmul(ps_f1[:], wf1T_sb[:], x_bf[:],
                                 start=True, stop=True)
                h1 = wpool.tile([D, TI], BF16, name="h1")
                nc.scalar.activation(h1[:], ps_f1[:], AF.Gelu, bias=b_f1_sb[:])
                ps_f2 = ps_s.tile([D, TI], F32, name="ps_f2", tag="s")
                nc.tensor.matmul(ps_f2[:], wf2T_sb[:], h1[:],
                                 start=True, stop=not with_bias)
                if with_bias:
                    nc.tensor.matmul(ps_f2[:], b_f2_sb[:], ones_row[:],
                                     start=False, stop=True)
                o = wpool.tile([D, TI], F32, name="o")
                nc.vector.tensor_add(o[:], ps_f2[:], x[:])
                nc.sync.dma_start(out[:, i * TI:(i + 1) * TI], o[:])

            xs = [None] * NT
            qpair, rpair = conv_mm0()
            prime = s_pair(0, qpair)
            for p in range(NP):
                last = p == NP - 1
                ps_a0 = ps_av.tile([D, TI], F32, name="ps_a0", tag="av")
                ps_a1 = ps_av.tile([D, TI], F32, name="ps_a1", tag="av")
                ps_m0 = ps_cm.tile([1, TI], F32, name="ps_m0", tag="m",
                                   bufs=2)
                ps_m1 = ps_cm.tile([1, TI], F32, name="ps_m1", tag="m",
                                   bufs=2)
                qn, rn = [None, None], [None, None]
                work = conv_work(p + 1, qn, rn) if not last else []
                wi = 0

                ps_sj = prime
                for jp in range(NJP):
                    # e[p, ii, t, jj]: jj innermost so the DoubleRow rhs
                    # streams byte-interleaved k-tile pairs (fast path)
                    e = epool.tile([128, 2, TI, 2], F8, name="e")
                    for jj in range(2):
                        j = 2 * jp + jj
                        ps_nxt = s_pair(j + 1, qpair) if j + 1 < NJ else None
                        for ii in range(2):
                            nc.scalar.activation(e[:, ii, :, jj],
                                                 ps_sj[ii][:], AF.Exp,
                                                 scale=SCALE, bias=ebias[:])
                        ps_sj = ps_nxt
                        # fill the exp wait with next pair's conv matmuls
                        budget = 2 if jj == 0 else 3
                        while budget > 0 and wi < len(work):
                            work[wi]()
                            wi += 1
                            budget -= 1
                    rhs0 = e[:, 0].rearrange("p t j -> p j t")
                    rhs1 = e[:, 1].rearrange("p t j -> p j t")
                    # AV pair back-to-back: both share the vT ldweights
                    for ps_aa, rhs in ((ps_a0, rhs0), (ps_a1, rhs1)):
                        nc.tensor.matmul(ps_aa[:],
                                         vT_sb[:, 2 * jp:2 * jp + 2, :],
                                         rhs, start=(jp == 0),
                                         stop=(jp == NJP - 1), perf_mode=DR)
                    for ps_mm, rhs in ((ps_m0, rhs0), (ps_m1, rhs1)):
                        nc.tensor.matmul(ps_mm[:], ones8[:, :, 0:1],
                                         rhs,
                                         start=(jp == 0), stop=(jp == NJP - 1),
                                         perf_mode=DR)
                while wi < len(work):
                    work[wi]()
                    wi += 1

                attn0 = normalize(0, ps_m0, ps_a0)
                if last:
                    ffn(xs[0], 0)
                    ffn(xs[1], 1)
                xs[2 * p] = proj(attn0, rpair[0], qpair[0])
                if not last:
                    qpair_n = [qn[0], qn[1]]
                    prime = s_pair(0, qpair_n)
                attn1 = normalize(1, ps_m1, ps_a1)
                if last:
                    ffn(xs[2], 2)
                    ffn(xs[3], 3)
                xs[2 * p + 1] = proj(attn1, rpair[1], qpair[1])
                if not last:
                    qpair, rpair = qpair_n, [rn[0], rn[1]]

            # ---- FFN for the last pair ----
            for i in range(2 * (NP - 1), NT):
                ffn(xs[i], i)

    _split_multi_waits(nc)
    return nc


_NC = {}


def _get_nc(with_bias=True):
    if with_bias not in _NC:
        _NC[with_bias] = build_nc(with_bias)
    return _NC[with_bias]


def _prep_core(corr, k, v, w_sk, b_sk, w_proj, b_proj, w_ffn1, b_ffn1,
               w_ffn2, b_ffn2):
    bf = ml_dtypes.bfloat16
    f8 = ml_dtypes.float8_e4m3
    wskT = np.empty((KC, 27, D), dtype=bf)
    for c in range(3):
        for t in range(9):
            dy, dx = t // 3, t % 3
            wskT[:, c * 9 + t, :] = \
                w_sk[:, c * KC:(c + 1) * KC, dy, dx].T.astype(bf)
    vT = v.reshape(D, HW).T.reshape(NJ, 128, D).transpose(1, 0, 2)
    return {
        "corr": corr.reshape(CIN, HW).astype(bf),
        "k": k.reshape(D, HW).astype(bf),
        "vT": np.ascontiguousarray(vT).astype(f8),
        "wskT": wskT,
        "b_sk": b_sk.reshape(1, D).astype(bf),
        "wprojT": np.ascontiguousarray(
            w_proj.reshape(D, 2 * D).T.reshape(2, D, D)).astype(bf),
        "b_proj": b_proj.reshape(1, D).astype(bf),
        "wf1T": np.ascontiguousarray(w_ffn1.reshape(D, D).T).astype(bf),
        "b_f1": b_ffn1.reshape(D, 1).astype(np.float32),
        "wf2T": np.ascontiguousarray(w_ffn2.reshape(D, D).T).astype(bf),
        "b_f2": b_ffn2.reshape(1, D).astype(bf),
    }


def make_in_maps(corr, k, v, w_sk, b_sk, w_proj, b_proj, w_ffn1, b_ffn1,
                 w_ffn2, b_ffn2):
    corr = np.asarray(corr, dtype=np.float32)
    k = np.asarray(k, dtype=np.float32)
    v = np.asarray(v, dtype=np.float32)
    return [
        _prep_core(corr[i], k[i], v[i], np.asarray(w_sk, np.float32),
                   np.asarray(b_sk, np.float32),
                   np.asarray(w_proj, np.float32),
                   np.asarray(b_proj, np.float32),
                   np.asarray(w_ffn1, np.float32),
                   np.asarray(b_ffn1, np.float32),
                   np.asarray(w_ffn2, np.float32),
                   np.asarray(b_ffn2, np.float32))
        for i in range(N)
    ]


def kernel(corr, k, v, w_sk, b_sk, w_proj, b_proj, w_ffn1, b_ffn1,
           w_ffn2, b_ffn2):
    with_bias = bool(np.any(np.asarray(b_proj)) or np.any(np.asarray(b_ffn2))
                     or np.any(np.asarray(b_sk)))
    nc = _get_nc(with_bias)
    in_maps = make_in_maps(corr, k, v, w_sk, b_sk, w_proj, b_proj,
                           w_ffn1, b_ffn1, w_ffn2, b_ffn2)
    res = run_bass_kernel_spmd(nc, in_maps, list(range(N)))
    out = np.stack([res.results[i]["out"].reshape(D, H, W) for i in range(N)])
    return out.astype(np.float32)


# revision 9
# speedup vs baseline: 1.1036x; 1.1036x over previous
"""CostGlobalEncoder TRN2 kernel: conv3x3(324->128) + global HW x HW attention
+ proj + FFN, data-parallel over batch N=8 across 8 NeuronCores.

Self-contained: hardcodes shapes N=8, D=128, H=48, W=64 (HW=3072).

v2: fp8 e/vT with DoubleRow A@V + in-loop softmax denominators, conv
interleaved into the attention j-loop, exp(-ln(d)) reciprocal on scalar.
"""
import sys
sys.path.insert(0, '/opt/trn_rl_repo')

import numpy as np
import ml_dtypes

import concourse.bass as bass
import concourse.tile as tile
from concourse import mybir
from concourse.bass_utils import run_bass_kernel_spmd

N, D, H, W = 8, 128, 48, 64
HW = H * W                    # 3072
CIN = 324                     # corr channels
KC = 108                      # conv contraction chunk (324 = 3*108)
NT = 6                        # i-tiles of 512 positions
NP = NT // 2                  # i-tile pairs
TI = 512                      # positions per i-tile
RT = TI // W                  # 8 rows per i-tile
NJ = HW // 128                # 24 j-tiles
NJP = NJ // 2                 # 12 j-tile pairs (fp8 DoubleRow)
SCALE = float(D) ** -0.5
EXP_BIAS = -2.5               # exp shift; cancels in softmax normalization

F32 = mybir.dt.float32
BF16 = mybir.dt.bfloat16
F8 = mybir.dt.float8e4
AF = mybir.ActivationFunctionType
DR = mybir.MatmulPerfMode.DoubleRow


def _split_multi_waits(nc, max_waits=1):
    """walrus setupSyncWait rejects instructions with several sem-waits;
    hoist extras onto preceding same-engine NOPs (engines run in order)."""
    for fn in nc.m.functions:
        for blk in fn.blocks:
            insts = blk.instructions
            i = 0
            while i < len(insts):
                inst = insts[i]
                si = inst.sync_info
                if si is not None and si.on_wait and len(si.on_wait) > max_waits:
                    waits = list(si.on_wait)
                    extra, keep = waits[:-max_waits], waits[-max_waits:]
                    nops = []
                    while extra:
                        chunk, extra = extra[:max_waits], extra[max_waits:]
                        nop = mybir.InstNoOp(
                            name=f"waitsplit-{nc.next_id()}", ins=[], outs=[])
                        nop.engine = inst.engine
                        nop.sync_info = mybir.SyncInfo(on_wait=chunk, on_update=[])
                        nops.append(nop)
                    inst.sync_info = mybir.SyncInfo(
                        on_wait=keep, on_update=list(si.on_update))
                    blk.instructions = insts = insts[:i] + nops + insts[i:]
                    i += len(nops)
                i += 1


def build_nc(with_bias=True):
    nc = bass.Bass()
    corr = nc.declare_dram_parameter("corr", [CIN, HW], BF16, isOutput=False)
    k_in = nc.declare_dram_parameter("k", [D, HW], BF16, isOutput=False)
    vT = nc.declare_dram_parameter("vT", [128, NJ, D], F8, isOutput=False)
    wskT = nc.declare_dram_parameter("wskT", [KC, 27, D], BF16, isOutput=False)
    b_sk = nc.declare_dram_parameter("b_sk", [1, D], BF16, isOutput=False)
    wprojT = nc.declare_dram_parameter("wprojT", [2, D, D], BF16, isOutput=False)
    b_proj = nc.declare_dram_parameter("b_proj", [1, D], BF16, isOutput=False)
    wf1T = nc.declare_dram_parameter("wf1T", [D, D], BF16, isOutput=False)
    b_f1 = nc.declare_dram_parameter("b_f1", [D, 1], F32, isOutput=False)
    wf2T = nc.declare_dram_parameter("wf2T", [D, D], BF16, isOutput=False)
    b_f2 = nc.declare_dram_parameter("b_f2", [1, D], BF16, isOutput=False)
    out = nc.declare_dram_parameter("out", [D, HW], F32, isOutput=True)

    with tile.TileContext(nc) as tc:
        with (
            tc.tile_pool(name="const", bufs=1) as cpool,
            tc.tile_pool(name="work", bufs=2) as wpool,
            tc.tile_pool(name="qpool", bufs=8) as qpool,
            tc.tile_pool(name="xpool", bufs=8) as xpool,
            tc.tile_pool(name="epool", bufs=4) as epool,
            tc.tile_pool(name="ps_s", bufs=3, space="PSUM") as ps_s,
            tc.tile_pool(name="ps_av", bufs=2, space="PSUM") as ps_av,
            tc.tile_pool(name="ps_cm", bufs=1, space="PSUM") as ps_cm,
        ):
            # ---- HAM warm-up first: PE busy from the very start so the
            # clock is at 2.4 GHz when the first conv runs ----
            warm = cpool.tile([128, 128], BF16)
            nc.vector.memset(warm[:], 0.0)
            ps_w = ps_cm.tile([128, 128], F32, name="ps_w", tag="c")
            for _ in range(90):
                nc.tensor.matmul(ps_w[:], warm[:], warm[:],
                                 start=True, stop=True)

            # ---- inputs: corr chunks staged on three parallel DMA
            # queues (sync/scalar/gpsimd), k split in halves ----
            corr_pad = []
            stgs = []
            for c in range(3):
                cp = cpool.tile([KC, H + 2, W + 2], BF16, name=f"corr_pad{c}")
                nc.vector.memset(cp[:, 0, :], 0.0)
                nc.vector.memset(cp[:, H + 1, :], 0.0)
                nc.vector.memset(cp[:, 1:H + 1, 0:1], 0.0)
                nc.vector.memset(cp[:, 1:H + 1, W + 1:W + 2], 0.0)
                corr_pad.append(cp)
                stgs.append(cpool.tile([KC, HW], BF16, name=f"stg{c}"))
            wskT_sb = cpool.tile([KC, 27, D], BF16)
            for c in range(3):
                nc.gpsimd.dma_start(wskT_sb[:, c * 9:(c + 1) * 9, :],
                                    wskT[:, c * 9:(c + 1) * 9, :])
            nc.sync.dma_start(stgs[0][:], corr[0 * KC:1 * KC, :])
            nc.scalar.dma_start(stgs[1][:], corr[1 * KC:2 * KC, :])
            nc.gpsimd.dma_start(stgs[2][:], corr[2 * KC:3 * KC, :])
            for c in range(3):
                nc.vector.tensor_copy(
                    corr_pad[c][:, 1:H + 1, 1:W + 1],
                    stgs[c].rearrange("p (h w) -> p h w", h=H))
            k_sb = cpool.tile([D, HW], BF16)
            nc.sync.dma_start(k_sb[:, 0:HW // 2], k_in[:, 0:HW // 2])
            nc.scalar.dma_start(k_sb[:, HW // 2:], k_in[:, HW // 2:])
            # vT_sb[p, t, d] = v[d, t*128+p], fp8
            vT_sb = cpool.tile([128, NJ, D], F8)
            nc.scalar.dma_start(vT_sb[:], vT[:])
            b_sk_sb = cpool.tile([1, D], BF16)
            nc.gpsimd.dma_start(b_sk_sb[:], b_sk[:])
            wprojT_sb = cpool.tile([D, 2, D], BF16)
            nc.gpsimd.dma_start(wprojT_sb[:], wprojT.rearrange("c p d -> p c d"))
            wf1T_sb = cpool.tile([D, D], BF16)
            nc.gpsimd.dma_start(wf1T_sb[:], wf1T[:])
            wf2T_sb = cpool.tile([D, D], BF16)
            nc.gpsimd.dma_start(wf2T_sb[:], wf2T[:])
            b_proj_sb = cpool.tile([1, D], BF16)
            nc.gpsimd.dma_start(b_proj_sb[:], b_proj[:])
            b_f1_sb = cpool.tile([D, 1], F32)
            nc.gpsimd.dma_start(b_f1_sb[:], b_f1[:])
            b_f2_sb = cpool.tile([1, D], BF16)
            nc.gpsimd.dma_start(b_f2_sb[:], b_f2[:])
            ones_row = cpool.tile([1, TI], BF16)
            nc.gpsimd.memset(ones_row[:], 1.0)
            ones1 = cpool.tile([1, 128], BF16)
            nc.gpsimd.memset(ones1[:], 1.0)
            # fp8 DoubleRow lhsT: k-pair stride must be 16B-aligned
            ones8 = cpool.tile([128, 2, 16], F8)
            nc.gpsimd.memset(ones8[:], 1.0)
            ebias = cpool.tile([128, 1], F32)
            nc.gpsimd.memset(ebias[:], EXP_BIAS)

            def conv_mm0():
                """conv for i-tiles (0, 1) pre-loop, in the av-pool slots.
                c-outer so matmuls start as soon as chunk 0 lands."""
                ps_cs = [ps_av.tile([D, TI], F32, name=f"ps_c0{ii}",
                                    tag="av") for ii in range(2)]
                for c in range(3):
                    for ii in range(2):
                        for t in range(9):
                            dy, dx = t // 3, t % 3
                            y0 = ii * RT
                            nc.tensor.matmul(
                                ps_cs[ii][:],
                                wskT_sb[:, c * 9 + t, :],
                                corr_pad[c][:, y0 + dy:y0 + dy + RT,
                                            dx:dx + W],
                                start=(c == 0 and t == 0),
                                stop=(c == 2 and t == 8 and not with_bias))
                qs, rs = [], []
                for ii in range(2):
                    if with_bias:
                        nc.tensor.matmul(ps_cs[ii][:], b_sk_sb[:],
                                         ones_row[:], start=False, stop=True)
                    q = qpool.tile([D, TI], BF16, name="q")
                    nc.vector.tensor_copy(q[:], ps_cs[ii][:])
                    qs.append(q)
                    resid = qpool.tile([D, TI], F32, name="resid")
                    nc.vector.tensor_copy(resid[:], ps_cs[ii][:])
                    rs.append(resid)
                return qs, rs

            def conv_work(pnext, qn, rn):
                """Closures computing pair pnext's conv, one i-tile at a
                time in the 1-buf "c" PSUM slot; results land in qn/rn."""
                work = []
                st = {}

                def mk_mm(ii, c, t):
                    def f():
                        if st.get('ii') != ii:
                            st['ps'] = ps_cm.tile([D, TI], F32,
                                                  name="ps_c", tag="c")
                            st['ii'] = ii
                        dy, dx = t // 3, t % 3
                        y0 = (2 * pnext + ii) * RT
                        nc.tensor.matmul(
                            st['ps'][:], wskT_sb[:, c * 9 + t, :],
                            corr_pad[c][:, y0 + dy:y0 + dy + RT, dx:dx + W],
                            start=(c == 0 and t == 0),
                            stop=(c == 2 and t == 8 and not with_bias))
                    return f

                def mk_bias(ii):
                    def f():
                        nc.tensor.matmul(st['ps'][:], b_sk_sb[:], ones_row[:],
                                         start=False, stop=True)
                    return f

                def mk_evac(ii):
                    def f():
                        q = qpool.tile([D, TI], BF16, name="q")
                        nc.vector.tensor_copy(q[:], st['ps'][:])
                        qn[ii] = q
                        resid = qpool.tile([D, TI], F32, name="resid")
                        nc.vector.tensor_copy(resid[:], st['ps'][:])
                        rn[ii] = resid
                    return f

                for ii in range(2):
                    for c in range(3):
                        for t in range(9):
                            work.append(mk_mm(ii, c, t))
                    if with_bias:
                        work.append(mk_bias(ii))
                    work.append(mk_evac(ii))
                return work

            def s_pair(j, qs):
                ts = []
                for ii in range(2):
                    t = ps_s.tile([128, TI], F32, name="ps_sj", tag="s")
                    nc.tensor.matmul(t[:],
                                     k_sb[:, j * 128:(j + 1) * 128],
                                     qs[ii][:], start=True, stop=True)
                    ts.append(t)
                return ts

            def normalize(ii, ps_m, ps_aa):
                """1/denominator = exp(-ln(d)) on the scalar engine (idle
                post-loop; ln+exp share an ACT table), broadcast via PE."""
                ln_row = wpool.tile([1, TI], F32, name="ln_row")
                nc.scalar.activation(ln_row[:], ps_m[:], AF.Ln)
                rrow = wpool.tile([1, TI], BF16, name="rrow")
                nc.scalar.activation(rrow[:], ln_row[:], AF.Exp, scale=-1.0)
                ps_b = ps_s.tile([128, TI], F32, name="ps_b", tag="s")
                nc.tensor.matmul(ps_b[:], ones1[:], rrow[:],
                                 start=True, stop=True)
                rb = wpool.tile([128, TI], BF16, name="rb")
                nc.scalar.copy(rb[:], ps_b[:])
                attn = wpool.tile([D, TI], BF16, name="attn")
                nc.vector.tensor_mul(attn[:], ps_aa[:], rb[:])
                return attn

            def proj(attn, resid, q):
                """1x1 proj on concat([attn, resid]) + bias + resid."""
                ps_p = ps_av.tile([D, TI], F32, name="ps_p", tag="av")
                nc.tensor.matmul(ps_p[:], wprojT_sb[:, 0, :], attn[:],
                                 start=True, stop=False)
                nc.tensor.matmul(ps_p[:], wprojT_sb[:, 1, :], q[:],
                                 start=False, stop=not with_bias)
                if with_bias:
                    nc.tensor.matmul(ps_p[:], b_proj_sb[:], ones_row[:],
                                     start=False, stop=True)
                x = xpool.tile([D, TI], F32, name="x")
                nc.vector.tensor_add(x[:], ps_p[:], resid[:])
                x_bf = xpool.tile([D, TI], BF16, name="x_bf")
                nc.vector.tensor_copy(x_bf[:], x[:])
                return x, x_bf

            def ffn(xv, i):
                x, x_bf = xv
                ps_f1 = ps_s.tile([D, TI], F32, name="ps_f1", tag="s")
                nc.tensor.matmul(ps_f1[:], wf1T_sb[:], x_bf[:],
                                 start=True, stop=True)
                h1 = wpool.tile([D, TI], BF16, name="h1")
                nc.scalar.activation(h1[:], ps_f1[:], AF.Gelu, bias=b_f1_sb[:])
                ps_f2 = ps_s.tile([D, TI], F32, name="ps_f2", tag="s")
                nc.tensor.matmul(ps_f2[:], wf2T_sb[:], h1[:],
                                 start=True, stop=not with_bias)
                if with_bias:
                    nc.tensor.matmul(ps_f2[:], b_f2_sb[:], ones_row[:],
                                     start=False, stop=True)
                o = wpool.tile([D, TI], F32, name="o")
                nc.vector.tensor_add(o[:], ps_f2[:], x[:])
                nc.sync.dma_start(out[:, i * TI:(i + 1) * TI], o[:])

            xs = [None] * NT
            qpair, rpair = conv_mm0()
            prime = s_pair(0, qpair)
            for p in range(NP):
                last = p == NP - 1
                ps_a0 = ps_av.tile([D, TI], F32, name="ps_a0", tag="av")
                ps_a1 = ps_av.tile([D, TI], F32, name="ps_a1", tag="av")
                ps_m0 = ps_cm.tile([1, TI], F32, name="ps_m0", tag="m",
                                   bufs=2)
                ps_m1 = ps_cm.tile([1, TI], F32, name="ps_m1", tag="m",
                                   bufs=2)
                qn, rn = [None, None], [None, None]
                work = conv_work(p + 1, qn, rn) if not last else []
                wi = 0

                ps_sj = prime
                for jp in range(NJP):
                    # e[p, ii, t, jj]: jj innermost so the DoubleRow rhs
                    # streams byte-interleaved k-tile pairs (fast path)
                    e = epool.tile([128, 2, TI, 2], F8, name="e")
                    for jj in range(2):
                        j = 2 * jp + jj
                        ps_nxt = s_pair(j + 1, qpair) if j + 1 < NJ else None
                        for ii in range(2):
                            nc.scalar.activation(e[:, ii, :, jj],
                                                 ps_sj[ii][:], AF.Exp,
                                                 scale=SCALE, bias=ebias[:])
                        ps_sj = ps_nxt
                        # fill the exp wait with next pair's conv matmuls
                        budget = 2 if jj == 0 else 3
                        while budget > 0 and wi < len(work):
                            work[wi]()
                            wi += 1
                            budget -= 1
                    rhs0 = e[:, 0].rearrange("p t j -> p j t")
                    rhs1 = e[:, 1].rearrange("p t j -> p j t")
                    # AV pair back-to-back: both share the vT ldweights
                    for ps_aa, rhs in ((ps_a0, rhs0), (ps_a1, rhs1)):
                        nc.tensor.matmul(ps_aa[:],
                                         vT_sb[:, 2 * jp:2 * jp + 2, :],
                                         rhs, start=(jp == 0),
                                         stop=(jp == NJP - 1), perf_mode=DR)
                    for ps_mm, rhs in ((ps_m0, rhs0), (ps_m1, rhs1)):
                        nc.tensor.matmul(ps_mm[:], ones8[:, :, 0:1],
                                         rhs,
                                         start=(jp == 0), stop=(jp == NJP - 1),
                                         perf_mode=DR)
                while wi < len(work):
                    work[wi]()
                    wi += 1

                attn0 = normalize(0, ps_m0, ps_a0)
                if last:
                    ffn(xs[0], 0)
                    ffn(xs[1], 1)
                xs[2 * p] = proj(attn0, rpair[0], qpair[0])
                if not last:
                    qpair_n = [qn[0], qn[1]]
                    prime = s_pair(0, qpair_n)
                attn1 = normalize(1, ps_m1, ps_a1)
                if last:
                    ffn(xs[2], 2)
                    ffn(xs[3], 3)
                xs[2 * p + 1] = proj(attn1, rpair[1], qpair[1])
                if not last:
                    qpair, rpair = qpair_n, [rn[0], rn[1]]

            # ---- FFN for the last pair ----
            for i in range(2 * (NP - 1), NT):
                ffn(xs[i], i)

    _split_multi_waits(nc)
    return nc


_NC = {}


def _get_nc(with_bias=True):
    if with_bias not in _NC:
        _NC[with_bias] = build_nc(with_bias)
    return _NC[with_bias]


def _prep_core(corr, k, v, w_sk, b_sk, w_proj, b_proj, w_ffn1, b_ffn1,
               w_ffn2, b_ffn2):
    bf = ml_dtypes.bfloat16
    f8 = ml_dtypes.float8_e4m3
    wskT = np.empty((KC, 27, D), dtype=bf)
    for c in range(3):
        for t in range(9):
            dy, dx = t // 3, t % 3
            wskT[:, c * 9 + t, :] = \
                w_sk[:, c * KC:(c + 1) * KC, dy, dx].T.astype(bf)
    vT = v.reshape(D, HW).T.reshape(NJ, 128, D).transpose(1, 0, 2)
    return {
        "corr": corr.reshape(CIN, HW).astype(bf),
        "k": k.reshape(D, HW).astype(bf),
        "vT": np.ascontiguousarray(vT).astype(f8),
        "wskT": wskT,
        "b_sk": b_sk.reshape(1, D).astype(bf),
        "wprojT": np.ascontiguousarray(
            w_proj.reshape(D, 2 * D).T.reshape(2, D, D)).astype(bf),
        "b_proj": b_proj.reshape(1, D).astype(bf),
        "wf1T": np.ascontiguousarray(w_ffn1.reshape(D, D).T).astype(bf),
        "b_f1": b_ffn1.reshape(D, 1).astype(np.float32),
        "wf2T": np.ascontiguousarray(w_ffn2.reshape(D, D).T).astype(bf),
        "b_f2": b_ffn2.reshape(1, D).astype(bf),
    }


def make_in_maps(corr, k, v, w_sk, b_sk, w_proj, b_proj, w_ffn1, b_ffn1,
                 w_ffn2, b_ffn2):
    corr = np.asarray(corr, dtype=np.float32)
    k = np.asarray(k, dtype=np.float32)
    v = np.asarray(v, dtype=np.float32)
    return [
        _prep_core(corr[i], k[i], v[i], np.asarray(w_sk, np.float32),
                   np.asarray(b_sk, np.float32),
                   np.asarray(w_proj, np.float32),
                   np.asarray(b_proj, np.float32),
                   np.asarray(w_ffn1, np.float32),
                   np.asarray(b_ffn1, np.float32),
                   np.asarray(w_ffn2, np.float32),
                   np.asarray(b_ffn2, np.float32))
        for i in range(N)
    ]


def kernel(corr, k, v, w_sk, b_sk, w_proj, b_proj, w_ffn1, b_ffn1,
           w_ffn2, b_ffn2):
    with_bias = bool(np.any(np.asarray(b_proj)) or np.any(np.asarray(b_ffn2))
                     or np.any(np.asarray(b_sk)))
    nc = _get_nc(with_bias)
    in_maps = make_in_maps(corr, k, v, w_sk, b_sk, w_proj, b_proj,
                           w_ffn1, b_ffn1, w_ffn2, b_ffn2)
    res = run_bass_kernel_spmd(nc, in_maps, list(range(N)))
    out = np.stack([res.results[i]["out"].reshape(D, H, W) for i in range(N)])
    return out.astype(np.float32)


# revision 12
# speedup vs baseline: 1.1268x; 1.0210x over previous
"""CostGlobalEncoder TRN2 kernel: conv3x3(324->128) + global HW x HW attention
+ proj + FFN, data-parallel over batch N=8 across 8 NeuronCores.

Self-contained: hardcodes shapes N=8, D=128, H=48, W=64 (HW=3072).

v2: fp8 e/vT with DoubleRow A@V + in-loop softmax denominators, conv
interleaved into the attention j-loop, exp(-ln(d)) reciprocal on scalar.
"""
import sys
sys.path.insert(0, '/opt/trn_rl_repo')

import numpy as np
import ml_dtypes

import concourse.bass as bass
import concourse.tile as tile
from concourse import mybir
from concourse.bass_utils import run_bass_kernel_spmd

N, D, H, W = 8, 128, 48, 64
HW = H * W                    # 3072
CIN = 324                     # corr channels
KC = 108                      # conv contraction chunk (324 = 3*108)
NT = 6                        # i-tiles of 512 positions
NP = NT // 2                  # i-tile pairs
TI = 512                      # positions per i-tile
RT = TI // W                  # 8 rows per i-tile
NJ = HW // 128                # 24 j-tiles
NJP = NJ // 2                 # 12 j-tile pairs (fp8 DoubleRow)
SCALE = float(D) ** -0.5
EXP_BIAS = -2.5               # exp shift; cancels in softmax normalization

F32 = mybir.dt.float32
BF16 = mybir.dt.bfloat16
F8 = mybir.dt.float8e4
AF = mybir.ActivationFunctionType
DR = mybir.MatmulPerfMode.DoubleRow


def _split_multi_waits(nc, max_waits=1):
    """walrus setupSyncWait rejects instructions with several sem-waits;
    hoist extras onto preceding same-engine NOPs (engines run in order)."""
    for fn in nc.m.functions:
        for blk in fn.blocks:
            insts = blk.instructions
            i = 0
            while i < len(insts):
                inst = insts[i]
                si = inst.sync_info
                if si is not None and si.on_wait and len(si.on_wait) > max_waits:
                    waits = list(si.on_wait)
                    extra, keep = waits[:-max_waits], waits[-max_waits:]
                    nops = []
                    while extra:
                        chunk, extra = extra[:max_waits], extra[max_waits:]
                        nop = mybir.InstNoOp(
                            name=f"waitsplit-{nc.next_id()}", ins=[], outs=[])
                        nop.engine = inst.engine
                        nop.sync_info = mybir.SyncInfo(on_wait=chunk, on_update=[])
                        nops.append(nop)
                    inst.sync_info = mybir.SyncInfo(
                        on_wait=keep, on_update=list(si.on_update))
                    blk.instructions = insts = insts[:i] + nops + insts[i:]
                    i += len(nops)
                i += 1


def build_nc(with_bias=True):
    nc = bass.Bass()
    # corr arrives host-pre-padded: [CIN, (H+2)*(W+2)]
    corr = nc.declare_dram_parameter("corr", [CIN, (H + 2) * (W + 2)], BF16,
                                     isOutput=False)
    k_in = nc.declare_dram_parameter("k", [D, HW], BF16, isOutput=False)
    vT = nc.declare_dram_parameter("vT", [128, NJ, D], F8, isOutput=False)
    wskT = nc.declare_dram_parameter("wskT", [KC, 27, D], BF16, isOutput=False)
    b_sk = nc.declare_dram_parameter("b_sk", [1, D], BF16, isOutput=False)
    wprojT = nc.declare_dram_parameter("wprojT", [2, D, D], BF16, isOutput=False)
    b_proj = nc.declare_dram_parameter("b_proj", [1, D], BF16, isOutput=False)
    wf1T = nc.declare_dram_parameter("wf1T", [D, D], BF16, isOutput=False)
    b_f1 = nc.declare_dram_parameter("b_f1", [D, 1], F32, isOutput=False)
    wf2T = nc.declare_dram_parameter("wf2T", [D, D], BF16, isOutput=False)
    b_f2 = nc.declare_dram_parameter("b_f2", [1, D], BF16, isOutput=False)
    out = nc.declare_dram_parameter("out", [D, HW], F32, isOutput=True)

    with tile.TileContext(nc) as tc:
        with (
            tc.tile_pool(name="const", bufs=1) as cpool,
            tc.tile_pool(name="work", bufs=2) as wpool,
            tc.tile_pool(name="qpool", bufs=8) as qpool,
            tc.tile_pool(name="xpool", bufs=8) as xpool,
            tc.tile_pool(name="epool", bufs=4) as epool,
            tc.tile_pool(name="ps_s", bufs=3, space="PSUM") as ps_s,
            tc.tile_pool(name="ps_av", bufs=2, space="PSUM") as ps_av,
            tc.tile_pool(name="ps_cm", bufs=1, space="PSUM") as ps_cm,
        ):
            # ---- HAM warm-up first: PE busy from the very start so the
            # clock is at 2.4 GHz when the first conv runs ----
            warm = cpool.tile([128, 128], BF16)
            nc.vector.memset(warm[:], 0.0)
            ps_w = ps_cm.tile([128, 128], F32, name="ps_w", tag="c")
            for _ in range(75):
                nc.tensor.matmul(ps_w[:], warm[:], warm[:],
                                 start=True, stop=True)

            # ---- inputs: corr is host-pre-padded, so chunks DMA
            # contiguously straight into SBUF, halves on two queues ----
            PADHW = (H + 2) * (W + 2)
            corr_pad = []
            for c in range(3):
                cp = cpool.tile([KC, H + 2, W + 2], BF16, name=f"corr_pad{c}")
                cpf = cp.rearrange("p h w -> p (h w)")
                src = corr[c * KC:(c + 1) * KC, :]
                nc.sync.dma_start(cpf[:, 0:PADHW // 2], src[:, 0:PADHW // 2])
                nc.scalar.dma_start(cpf[:, PADHW // 2:], src[:, PADHW // 2:])
                corr_pad.append(cp)
            wskT_sb = cpool.tile([KC, 27, D], BF16)
            for c in range(3):
                nc.gpsimd.dma_start(wskT_sb[:, c * 9:(c + 1) * 9, :],
                                    wskT[:, c * 9:(c + 1) * 9, :])
            k_sb = cpool.tile([D, HW], BF16)
            nc.sync.dma_start(k_sb[:, 0:HW // 2], k_in[:, 0:HW // 2])
            nc.scalar.dma_start(k_sb[:, HW // 2:], k_in[:, HW // 2:])
            # vT_sb[p, t, d] = v[d, t*128+p], fp8
            vT_sb = cpool.tile([128, NJ, D], F8)
            nc.gpsimd.dma_start(vT_sb[:], vT[:])
            b_sk_sb = cpool.tile([1, D], BF16)
            nc.gpsimd.dma_start(b_sk_sb[:], b_sk[:])
            wprojT_sb = cpool.tile([D, 2, D], BF16)
            nc.gpsimd.dma_start(wprojT_sb[:], wprojT.rearrange("c p d -> p c d"))
            wf1T_sb = cpool.tile([D, D], BF16)
            nc.gpsimd.dma_start(wf1T_sb[:], wf1T[:])
            wf2T_sb = cpool.tile([D, D], BF16)
            nc.gpsimd.dma_start(wf2T_sb[:], wf2T[:])
            b_proj_sb = cpool.tile([1, D], BF16)
            nc.gpsimd.dma_start(b_proj_sb[:], b_proj[:])
            b_f1_sb = cpool.tile([D, 1], F32)
            nc.gpsimd.dma_start(b_f1_sb[:], b_f1[:])
            b_f2_sb = cpool.tile([1, D], BF16)
            nc.gpsimd.dma_start(b_f2_sb[:], b_f2[:])
            ones_row = cpool.tile([1, TI], BF16)
            nc.gpsimd.memset(ones_row[:], 1.0)
            ones1 = cpool.tile([1, 128], BF16)
            nc.gpsimd.memset(ones1[:], 1.0)
            # fp8 DoubleRow lhsT: k-pair stride must be 16B-aligned
            ones8 = cpool.tile([128, 2, 16], F8)
            nc.gpsimd.memset(ones8[:], 1.0)
            ebias = cpool.tile([128, 1], F32)
            nc.gpsimd.memset(ebias[:], EXP_BIAS)

            def conv_mm0():
                """conv for i-tiles (0, 1) pre-loop, in the av-pool slots.
                c-outer so matmuls start as soon as chunk 0 lands."""
                ps_cs = [ps_av.tile([D, TI], F32, name=f"ps_c0{ii}",
                                    tag="av") for ii in range(2)]
                for c in range(3):
                    for ii in range(2):
                        for t in range(9):
                            dy, dx = t // 3, t % 3
                            y0 = ii * RT
                            nc.tensor.matmul(
                                ps_cs[ii][:],
                                wskT_sb[:, c * 9 + t, :],
                                corr_pad[c][:, y0 + dy:y0 + dy + RT,
                                            dx:dx + W],
                                start=(c == 0 and t == 0),
                                stop=(c == 2 and t == 8 and not with_bias))
                qs, rs = [], []
                for ii in range(2):
                    if with_bias:
                        nc.tensor.matmul(ps_cs[ii][:], b_sk_sb[:],
                                         ones_row[:], start=False, stop=True)
                    q = qpool.tile([D, TI], BF16, name="q")
                    nc.vector.tensor_copy(q[:], ps_cs[ii][:])
                    qs.append(q)
                    resid = qpool.tile([D, TI], F32, name="resid")
                    nc.vector.tensor_copy(resid[:], ps_cs[ii][:])
                    rs.append(resid)
                return qs, rs

            def conv_work(pnext, qn, rn):
                """Closures computing pair pnext's conv, one i-tile at a
                time in the 1-buf "c" PSUM slot; results land in qn/rn."""
                work = []
                st = {}

                def mk_mm(ii, c, t):
                    def f():
                        if st.get('ii') != ii:
                            st['ps'] = ps_cm.tile([D, TI], F32,
                                                  name="ps_c", tag="c")
                            st['ii'] = ii
                        dy, dx = t // 3, t % 3
                        y0 = (2 * pnext + ii) * RT
                        nc.tensor.matmul(
                            st['ps'][:], wskT_sb[:, c * 9 + t, :],
                            corr_pad[c][:, y0 + dy:y0 + dy + RT, dx:dx + W],
                            start=(c == 0 and t == 0),
                            stop=(c == 2 and t == 8 and not with_bias))
                    return f

                def mk_bias(ii):
                    def f():
                        nc.tensor.matmul(st['ps'][:], b_sk_sb[:], ones_row[:],
                                         start=False, stop=True)
                    return f

                def mk_evac(ii):
                    def f():
                        q = qpool.tile([D, TI], BF16, name="q")
                        nc.vector.tensor_copy(q[:], st['ps'][:])
                        qn[ii] = q
                        resid = qpool.tile([D, TI], F32, name="resid")
                        nc.vector.tensor_copy(resid[:], st['ps'][:])
                        rn[ii] = resid
                    return f

                for ii in range(2):
                    for c in range(3):
                        for t in range(9):
                            work.append(mk_mm(ii, c, t))
                    if with_bias:
                        work.append(mk_bias(ii))
                    work.append(mk_evac(ii))
                return work

            def s_pair(j, qs):
                ts = []
                for ii in range(2):
                    t = ps_s.tile([128, TI], F32, name="ps_sj", tag="s")
                    nc.tensor.matmul(t[:],
                                     k_sb[:, j * 128:(j + 1) * 128],
                                     qs[ii][:], start=True, stop=True)
                    ts.append(t)
                return ts

            def normalize(ii, ps_m, ps_aa):
                """1/denominator = exp(-ln(d)) on the scalar engine (idle
                post-loop; ln+exp share an ACT table), broadcast via PE."""
                ln_row = wpool.tile([1, TI], F32, name="ln_row")
                nc.scalar.activation(ln_row[:], ps_m[:], AF.Ln)
                rrow = wpool.tile([1, TI], BF16, name="rrow")
                nc.scalar.activation(rrow[:], ln_row[:], AF.Exp, scale=-1.0)
                ps_b = ps_s.tile([128, TI], F32, name="ps_b", tag="s")
                nc.tensor.matmul(ps_b[:], ones1[:], rrow[:],
                                 start=True, stop=True)
                rb = wpool.tile([128, TI], BF16, name="rb")
                nc.scalar.copy(rb[:], ps_b[:])
                attn = wpool.tile([D, TI], BF16, name="attn")
                nc.vector.tensor_mul(attn[:], ps_aa[:], rb[:])
                return attn

            def proj(attn, resid, q):
                """1x1 proj on concat([attn, resid]) + bias + resid."""
                ps_p = ps_av.tile([D, TI], F32, name="ps_p", tag="av")
                nc.tensor.matmul(ps_p[:], wprojT_sb[:, 0, :], attn[:],
                                 start=True, stop=False)
                nc.tensor.matmul(ps_p[:], wprojT_sb[:, 1, :], q[:],
                                 start=False, stop=not with_bias)
                if with_bias:
                    nc.tensor.matmul(ps_p[:], b_proj_sb[:], ones_row[:],
                                     start=False, stop=True)
                x = xpool.tile([D, TI], F32, name="x")
                nc.vector.tensor_add(x[:], ps_p[:], resid[:])
                x_bf = xpool.tile([D, TI], BF16, name="x_bf")
                nc.vector.tensor_copy(x_bf[:], x[:])
                return x, x_bf

            def ffn(xv, i):
                x, x_bf = xv
                ps_f1 = ps_s.tile([D, TI], F32, name="ps_f1", tag="s")
                nc.tensor.matmul(ps_f1[:], wf1T_sb[:], x_bf[:],
                                 start=True, stop=True)
                h1 = wpool.tile([D, TI], BF16, name="h1")
                nc.scalar.activation(h1[:], ps_f1[:], AF.Gelu, bias=b_f1_sb[:])
                ps_f2 = ps_s.tile([D, TI], F32, name="ps_f2", tag="s")
                nc.tensor.matmul(ps_f2[:], wf2T_sb[:], h1[:],
                                 start=True, stop=not with_bias)
                if with_bias:
                    nc.tensor.matmul(ps_f2[:], b_f2_sb[:], ones_row[:],
                                     start=False, stop=True)
                o = wpool.tile([D, TI], F32, name="o")
                nc.vector.tensor_add(o[:], ps_f2[:], x[:])
                nc.sync.dma_start(out[:, i * TI:(i + 1) * TI], o[:])

            xs = [None] * NT
            qpair, rpair = conv_mm0()
            prime = s_pair(0, qpair)
            for p in range(NP):
                last = p == NP - 1
                ps_a0 = ps_av.tile([D, TI], F32, name="ps_a0", tag="av")
                ps_a1 = ps_av.tile([D, TI], F32, name="ps_a1", tag="av")
                ps_m0 = ps_cm.tile([1, TI], F32, name="ps_m0", tag="m",
                                   bufs=2)
                ps_m1 = ps_cm.tile([1, TI], F32, name="ps_m1", tag="m",
                                   bufs=2)
                qn, rn = [None, None], [None, None]
                work = conv_work(p + 1, qn, rn) if not last else []
                wi = 0

                ps_sj = prime
                for jp in range(NJP):
                    # e[p, ii, t, jj]: jj innermost so the DoubleRow rhs
                    # streams byte-interleaved k-tile pairs (fast path)
                    e = epool.tile([128, 2, TI, 2], F8, name="e")
                    for jj in range(2):
                        j = 2 * jp + jj
                        ps_nxt = s_pair(j + 1, qpair) if j + 1 < NJ else None
                        for ii in range(2):
                            nc.scalar.activation(e[:, ii, :, jj],
                                                 ps_sj[ii][:], AF.Exp,
                                                 scale=SCALE, bias=ebias[:])
                        ps_sj = ps_nxt
                        # fill the exp wait with next pair's conv matmuls
                        budget = 2 if jj == 0 else 3
                        while budget > 0 and wi < len(work):
                            work[wi]()
                            wi += 1
                            budget -= 1
                    rhs0 = e[:, 0].rearrange("p t j -> p j t")
                    rhs1 = e[:, 1].rearrange("p t j -> p j t")
                    # AV pair back-to-back: both share the vT ldweights
                    for ps_aa, rhs in ((ps_a0, rhs0), (ps_a1, rhs1)):
                        nc.tensor.matmul(ps_aa[:],
                                         vT_sb[:, 2 * jp:2 * jp + 2, :],
                                         rhs, start=(jp == 0),
                                         stop=(jp == NJP - 1), perf_mode=DR)
                    for ps_mm, rhs in ((ps_m0, rhs0), (ps_m1, rhs1)):
                        nc.tensor.matmul(ps_mm[:], ones8[:, :, 0:1],
                                         rhs,
                                         start=(jp == 0), stop=(jp == NJP - 1),
                                         perf_mode=DR)
                while wi < len(work):
                    work[wi]()
                    wi += 1

                attn0 = normalize(0, ps_m0, ps_a0)
                if last:
                    ffn(xs[0], 0)
                    ffn(xs[1], 1)
                xs[2 * p] = proj(attn0, rpair[0], qpair[0])
                if not last:
                    qpair_n = [qn[0], qn[1]]
                    prime = s_pair(0, qpair_n)
                attn1 = normalize(1, ps_m1, ps_a1)
                if last:
                    ffn(xs[2], 2)
                    ffn(xs[3], 3)
                xs[2 * p + 1] = proj(attn1, rpair[1], qpair[1])
                if not last:
                    qpair, rpair = qpair_n, [rn[0], rn[1]]

            # ---- FFN for the last pair ----
            for i in range(2 * (NP - 1), NT):
                ffn(xs[i], i)

    _split_multi_waits(nc)
    return nc


_NC = {}


def _get_nc(with_bias=True):
    if with_bias not in _NC:
        _NC[with_bias] = build_nc(with_bias)
    return _NC[with_bias]


def _prep_core(corr, k, v, w_sk, b_sk, w_proj, b_proj, w_ffn1, b_ffn1,
               w_ffn2, b_ffn2):
    bf = ml_dtypes.bfloat16
    f8 = ml_dtypes.float8_e4m3
    wskT = np.empty((KC, 27, D), dtype=bf)
    for c in range(3):
        for t in range(9):
            dy, dx = t // 3, t % 3
            wskT[:, c * 9 + t, :] = \
                w_sk[:, c * KC:(c + 1) * KC, dy, dx].T.astype(bf)
    vT = v.reshape(D, HW).T.reshape(NJ, 128, D).transpose(1, 0, 2)
    corr_p = np.zeros((CIN, H + 2, W + 2), dtype=bf)
    corr_p[:, 1:H + 1, 1:W + 1] = corr.reshape(CIN, H, W)
    return {
        "corr": corr_p.reshape(CIN, (H + 2) * (W + 2)),
        "k": k.reshape(D, HW).astype(bf),
        "vT": np.ascontiguousarray(vT).astype(f8),
        "wskT": wskT,
        "b_sk": b_sk.reshape(1, D).astype(bf),
        "wprojT": np.ascontiguousarray(
            w_proj.reshape(D, 2 * D).T.reshape(2, D, D)).astype(bf),
        "b_proj": b_proj.reshape(1, D).astype(bf),
        "wf1T": np.ascontiguousarray(w_ffn1.reshape(D, D).T).astype(bf),
        "b_f1": b_ffn1.reshape(D, 1).astype(np.float32),
        "wf2T": np.ascontiguousarray(w_ffn2.reshape(D, D).T).astype(bf),
        "b_f2": b_ffn2.reshape(1, D).astype(bf),
    }


def make_in_maps(corr, k, v, w_sk, b_sk, w_proj, b_proj, w_ffn1, b_ffn1,
                 w_ffn2, b_ffn2):
    corr = np.asarray(corr, dtype=np.float32)
    k = np.asarray(k, dtype=np.float32)
    v = np.asarray(v, dtype=np.float32)
    return [
        _prep_core(corr[i], k[i], v[i], np.asarray(w_sk, np.float32),
                   np.asarray(b_sk, np.float32),
                   np.asarray(w_proj, np.float32),
                   np.asarray(b_proj, np.float32),
                   np.asarray(w_ffn1, np.float32),
                   np.asarray(b_ffn1, np.float32),
                   np.asarray(w_ffn2, np.float32),
                   np.asarray(b_ffn2, np.float32))
        for i in range(N)
    ]


def kernel(corr, k, v, w_sk, b_sk, w_proj, b_proj, w_ffn1, b_ffn1,
           w_ffn2, b_ffn2):
    with_bias = bool(np.any(np.asarray(b_proj)) or np.any(np.asarray(b_ffn2))
                     or np.any(np.asarray(b_sk)))
    nc = _get_nc(with_bias)
    in_maps = make_in_maps(corr, k, v, w_sk, b_sk, w_proj, b_proj,
                           w_ffn1, b_ffn1, w_ffn2, b_ffn2)
    res = run_bass_kernel_spmd(nc, in_maps, list(range(N)))
    out = np.stack([res.results[i]["out"].reshape(D, H, W) for i in range(N)])
    return out.astype(np.float32)


# revision 15
# speedup vs baseline: 1.1284x; 1.0014x over previous
"""CostGlobalEncoder TRN2 kernel: conv3x3(324->128) + global HW x HW attention
+ proj + FFN, data-parallel over batch N=8 across 8 NeuronCores.

Self-contained: hardcodes shapes N=8, D=128, H=48, W=64 (HW=3072).

v2: fp8 e/vT with DoubleRow A@V + in-loop softmax denominators, conv
interleaved into the attention j-loop, exp(-ln(d)) reciprocal on scalar.
"""
import sys
sys.path.insert(0, '/opt/trn_rl_repo')

import numpy as np
import ml_dtypes

import concourse.bass as bass
import concourse.tile as tile
from concourse import mybir
from concourse.bass_utils import run_bass_kernel_spmd

N, D, H, W = 8, 128, 48, 64
HW = H * W                    # 3072
CIN = 324                     # corr channels
KC = 108                      # conv contraction chunk (324 = 3*108)
NT = 6                        # i-tiles of 512 positions
NP = NT // 2                  # i-tile pairs
TI = 512                      # positions per i-tile
RT = TI // W                  # 8 rows per i-tile
NJ = HW // 128                # 24 j-tiles
NJP = NJ // 2                 # 12 j-tile pairs (fp8 DoubleRow)
SCALE = float(D) ** -0.5
EXP_BIAS = -2.5               # exp shift; cancels in softmax normalization

F32 = mybir.dt.float32
BF16 = mybir.dt.bfloat16
F8 = mybir.dt.float8e4
AF = mybir.ActivationFunctionType
DR = mybir.MatmulPerfMode.DoubleRow


def _split_multi_waits(nc, max_waits=1):
    """walrus setupSyncWait rejects instructions with several sem-waits;
    hoist extras onto preceding same-engine NOPs (engines run in order)."""
    for fn in nc.m.functions:
        for blk in fn.blocks:
            insts = blk.instructions
            i = 0
            while i < len(insts):
                inst = insts[i]
                si = inst.sync_info
                if si is not None and si.on_wait and len(si.on_wait) > max_waits:
                    waits = list(si.on_wait)
                    extra, keep = waits[:-max_waits], waits[-max_waits:]
                    nops = []
                    while extra:
                        chunk, extra = extra[:max_waits], extra[max_waits:]
                        nop = mybir.InstNoOp(
                            name=f"waitsplit-{nc.next_id()}", ins=[], outs=[])
                        nop.engine = inst.engine
                        nop.sync_info = mybir.SyncInfo(on_wait=chunk, on_update=[])
                        nops.append(nop)
                    inst.sync_info = mybir.SyncInfo(
                        on_wait=keep, on_update=list(si.on_update))
                    blk.instructions = insts = insts[:i] + nops + insts[i:]
                    i += len(nops)
                i += 1


def build_nc(with_bias=True):
    nc = bass.Bass()
    # corr arrives host-pre-padded: [CIN, (H+2)*(W+2)]
    corr = nc.declare_dram_parameter("corr", [CIN, (H + 2) * (W + 2)], BF16,
                                     isOutput=False)
    k_in = nc.declare_dram_parameter("k", [D, HW], BF16, isOutput=False)
    vT = nc.declare_dram_parameter("vT", [128, NJ, D], F8, isOutput=False)
    wskT = nc.declare_dram_parameter("wskT", [KC, 27, D], BF16, isOutput=False)
    b_sk = nc.declare_dram_parameter("b_sk", [1, D], BF16, isOutput=False)
    wprojT = nc.declare_dram_parameter("wprojT", [2, D, D], BF16, isOutput=False)
    b_proj = nc.declare_dram_parameter("b_proj", [1, D], BF16, isOutput=False)
    wf1T = nc.declare_dram_parameter("wf1T", [D, D], BF16, isOutput=False)
    b_f1 = nc.declare_dram_parameter("b_f1", [D, 1], F32, isOutput=False)
    wf2T = nc.declare_dram_parameter("wf2T", [D, D], BF16, isOutput=False)
    b_f2 = nc.declare_dram_parameter("b_f2", [1, D], BF16, isOutput=False)
    out = nc.declare_dram_parameter("out", [D, HW], F32, isOutput=True)

    with tile.TileContext(nc) as tc:
        with (
            tc.tile_pool(name="const", bufs=1) as cpool,
            tc.tile_pool(name="work", bufs=2) as wpool,
            tc.tile_pool(name="qpool", bufs=8) as qpool,
            tc.tile_pool(name="xpool", bufs=8) as xpool,
            tc.tile_pool(name="epool", bufs=14) as epool,
            tc.tile_pool(name="ps_s", bufs=2, space="PSUM") as ps_s,
            tc.tile_pool(name="ps_av", bufs=2, space="PSUM") as ps_av,
            tc.tile_pool(name="ps_cm", bufs=1, space="PSUM") as ps_cm,
        ):
            # ---- HAM warm-up first: PE busy from the very start so the
            # clock is at 2.4 GHz when the first conv runs ----
            warm = cpool.tile([128, 128], BF16)
            nc.vector.memset(warm[:], 0.0)
            ps_w = ps_cm.tile([128, 128], F32, name="ps_w", tag="c")
            for _ in range(75):
                nc.tensor.matmul(ps_w[:], warm[:], warm[:],
                                 start=True, stop=True)

            # ---- inputs: corr is host-pre-padded, so chunks DMA
            # contiguously straight into SBUF, halves on two queues ----
            PADHW = (H + 2) * (W + 2)
            corr_pad = []
            for c in range(3):
                cp = cpool.tile([KC, H + 2, W + 2], BF16, name=f"corr_pad{c}")
                cpf = cp.rearrange("p h w -> p (h w)")
                src = corr[c * KC:(c + 1) * KC, :]
                nc.sync.dma_start(cpf[:, 0:PADHW // 2], src[:, 0:PADHW // 2])
                nc.scalar.dma_start(cpf[:, PADHW // 2:], src[:, PADHW // 2:])
                corr_pad.append(cp)
            wskT_sb = cpool.tile([KC, 27, D], BF16)
            for c in range(3):
                nc.gpsimd.dma_start(wskT_sb[:, c * 9:(c + 1) * 9, :],
                                    wskT[:, c * 9:(c + 1) * 9, :])
            k_sb = cpool.tile([D, HW], BF16)
            nc.sync.dma_start(k_sb[:, 0:HW // 2], k_in[:, 0:HW // 2])
            nc.scalar.dma_start(k_sb[:, HW // 2:], k_in[:, HW // 2:])
            # vT_sb[p, t, d] = v[d, t*128+p], fp8
            vT_sb = cpool.tile([128, NJ, D], F8)
            nc.gpsimd.dma_start(vT_sb[:], vT[:])
            b_sk_sb = cpool.tile([1, D], BF16)
            nc.gpsimd.dma_start(b_sk_sb[:], b_sk[:])
            wprojT_sb = cpool.tile([D, 2, D], BF16)
            nc.gpsimd.dma_start(wprojT_sb[:], wprojT.rearrange("c p d -> p c d"))
            wf1T_sb = cpool.tile([D, D], BF16)
            nc.gpsimd.dma_start(wf1T_sb[:], wf1T[:])
            wf2T_sb = cpool.tile([D, D], BF16)
            nc.gpsimd.dma_start(wf2T_sb[:], wf2T[:])
            b_proj_sb = cpool.tile([1, D], BF16)
            nc.gpsimd.dma_start(b_proj_sb[:], b_proj[:])
            b_f1_sb = cpool.tile([D, 1], F32)
            nc.gpsimd.dma_start(b_f1_sb[:], b_f1[:])
            b_f2_sb = cpool.tile([1, D], BF16)
            nc.gpsimd.dma_start(b_f2_sb[:], b_f2[:])
            ones_row = cpool.tile([1, TI], BF16)
            nc.gpsimd.memset(ones_row[:], 1.0)
            ones1 = cpool.tile([1, 128], BF16)
            nc.gpsimd.memset(ones1[:], 1.0)
            # fp8 DoubleRow lhsT: k-pair stride must be 16B-aligned
            ones8 = cpool.tile([128, 2, 16], F8)
            nc.gpsimd.memset(ones8[:], 1.0)
            ebias = cpool.tile([128, 1], F32)
            nc.gpsimd.memset(ebias[:], EXP_BIAS)

            def conv_mm0():
                """conv for i-tiles (0, 1) pre-loop, in the av-pool slots.
                c-outer so matmuls start as soon as chunk 0 lands."""
                ps_cs = [ps_av.tile([D, TI], F32, name=f"ps_c0{ii}",
                                    tag="av") for ii in range(2)]
                for c in range(3):
                    for ii in range(2):
                        for t in range(9):
                            dy, dx = t // 3, t % 3
                            y0 = ii * RT
                            nc.tensor.matmul(
                                ps_cs[ii][:],
                                wskT_sb[:, c * 9 + t, :],
                                corr_pad[c][:, y0 + dy:y0 + dy + RT,
                                            dx:dx + W],
                                start=(c == 0 and t == 0),
                                stop=(c == 2 and t == 8 and not with_bias))
                qs, rs = [], []
                for ii in range(2):
                    if with_bias:
                        nc.tensor.matmul(ps_cs[ii][:], b_sk_sb[:],
                                         ones_row[:], start=False, stop=True)
                    q = qpool.tile([D, TI], BF16, name="q")
                    nc.vector.tensor_copy(q[:], ps_cs[ii][:])
                    qs.append(q)
                    resid = qpool.tile([D, TI], F32, name="resid")
                    nc.vector.tensor_copy(resid[:], ps_cs[ii][:])
                    rs.append(resid)
                return qs, rs

            def conv_work(pnext, qn, rn):
                """Closures computing pair pnext's conv, one i-tile at a
                time in the 1-buf "c" PSUM slot; results land in qn/rn."""
                work = []
                st = {}

                def mk_mm(ii, c, t):
                    def f():
                        if st.get('ii') != ii:
                            st['ps'] = ps_cm.tile([D, TI], F32,
                                                  name="ps_c", tag="c")
                            st['ii'] = ii
                        dy, dx = t // 3, t % 3
                        y0 = (2 * pnext + ii) * RT
                        nc.tensor.matmul(
                            st['ps'][:], wskT_sb[:, c * 9 + t, :],
                            corr_pad[c][:, y0 + dy:y0 + dy + RT, dx:dx + W],
                            start=(c == 0 and t == 0),
                            stop=(c == 2 and t == 8 and not with_bias))
                    return f

                def mk_bias(ii):
                    def f():
                        nc.tensor.matmul(st['ps'][:], b_sk_sb[:], ones_row[:],
                                         start=False, stop=True)
                    return f

                def mk_evac(ii):
                    def f():
                        q = qpool.tile([D, TI], BF16, name="q")
                        nc.vector.tensor_copy(q[:], st['ps'][:])
                        qn[ii] = q
                        resid = qpool.tile([D, TI], F32, name="resid")
                        nc.vector.tensor_copy(resid[:], st['ps'][:])
                        rn[ii] = resid
                    return f

                for ii in range(2):
                    for c in range(3):
                        for t in range(9):
                            work.append(mk_mm(ii, c, t))
                    if with_bias:
                        work.append(mk_bias(ii))
                    work.append(mk_evac(ii))
                return work

            def s_pair(j, qs):
                t = ps_s.tile([128, 2, TI], F32, name="ps_sj", tag="s")
                for ii in range(2):
                    nc.tensor.matmul(t[:, ii, :],
                                     k_sb[:, j * 128:(j + 1) * 128],
                                     qs[ii][:], start=True, stop=True)
                return t

            def normalize(ii, ps_m, ps_aa):
                """1/denominator = exp(-ln(d)) on the scalar engine (idle
                post-loop; ln+exp share an ACT table), broadcast via PE."""
                ln_row = wpool.tile([1, TI], F32, name="ln_row")
                nc.scalar.activation(ln_row[:], ps_m[:], AF.Ln)
                rrow = wpool.tile([1, TI], BF16, name="rrow")
                nc.scalar.activation(rrow[:], ln_row[:], AF.Exp, scale=-1.0)
                ps_b = ps_s.tile([128, TI], F32, name="ps_b", tag="s")
                nc.tensor.matmul(ps_b[:], ones1[:], rrow[:],
                                 start=True, stop=True)
                rb = wpool.tile([128, TI], BF16, name="rb")
                nc.scalar.copy(rb[:], ps_b[:])
                attn = wpool.tile([D, TI], BF16, name="attn")
                nc.vector.tensor_mul(attn[:], ps_aa[:], rb[:])
                return attn

            def proj(attn, resid, q):
                """1x1 proj on concat([attn, resid]) + bias + resid."""
                ps_p = ps_av.tile([D, TI], F32, name="ps_p", tag="av")
                nc.tensor.matmul(ps_p[:], wprojT_sb[:, 0, :], attn[:],
                                 start=True, stop=False)
                nc.tensor.matmul(ps_p[:], wprojT_sb[:, 1, :], q[:],
                                 start=False, stop=not with_bias)
                if with_bias:
                    nc.tensor.matmul(ps_p[:], b_proj_sb[:], ones_row[:],
                                     start=False, stop=True)
                x = xpool.tile([D, TI], F32, name="x")
                nc.vector.tensor_add(x[:], ps_p[:], resid[:])
                x_bf = xpool.tile([D, TI], BF16, name="x_bf")
                nc.vector.tensor_copy(x_bf[:], x[:])
                return x, x_bf

            def ffn(xv, i):
                x, x_bf = xv
                ps_f1 = ps_s.tile([D, TI], F32, name="ps_f1", tag="s")
                nc.tensor.matmul(ps_f1[:], wf1T_sb[:], x_bf[:],
                                 start=True, stop=True)
                h1 = wpool.tile([D, TI], BF16, name="h1")
                nc.scalar.activation(h1[:], ps_f1[:], AF.Gelu, bias=b_f1_sb[:])
                ps_f2 = ps_s.tile([D, TI], F32, name="ps_f2", tag="s")
                nc.tensor.matmul(ps_f2[:], wf2T_sb[:], h1[:],
                                 start=True, stop=not with_bias)
                if with_bias:
                    nc.tensor.matmul(ps_f2[:], b_f2_sb[:], ones_row[:],
                                     start=False, stop=True)
                o = wpool.tile([D, TI], F32, name="o")
                nc.vector.tensor_add(o[:], ps_f2[:], x[:])
                nc.sync.dma_start(out[:, i * TI:(i + 1) * TI], o[:])

            xs = [None] * NT
            qpair, rpair = conv_mm0()
            prime = s_pair(0, qpair)
            for p in range(NP):
                last = p == NP - 1
                ps_a0 = ps_av.tile([D, TI], F32, name="ps_a0", tag="av")
                ps_a1 = ps_av.tile([D, TI], F32, name="ps_a1", tag="av")
                ps_m0 = ps_cm.tile([1, TI], F32, name="ps_m0", tag="m")
                qn, rn = [None, None], [None, None]
                work = conv_work(p + 1, qn, rn) if not last else []
                wi = 0

                ps_sj = prime
                es = []
                for jp in range(NJP):
                    # e[p, ii, t, jj]: jj innermost so the DoubleRow rhs
                    # streams byte-interleaved k-tile pairs (fast path)
                    e = epool.tile([128, 2, TI, 2], F8, name="e")
                    for jj in range(2):
                        j = 2 * jp + jj
                        ps_nxt = s_pair(j + 1, qpair) if j + 1 < NJ else None
                        nc.scalar.activation(e[:, :, :, jj], ps_sj[:],
                                             AF.Exp, scale=SCALE,
                                             bias=ebias[:])
                        ps_sj = ps_nxt
                        # fill the exp wait with next pair's conv matmuls
                        budget = 2
                        while budget > 0 and wi < len(work):
                            work[wi]()
                            wi += 1
                            budget -= 1
                    rhs0 = e[:, 0].rearrange("p t j -> p j t")
                    rhs1 = e[:, 1].rearrange("p t j -> p j t")
                    for ps_aa, rhs in ((ps_a0, rhs0), (ps_a1, rhs1)):
                        nc.tensor.matmul(ps_aa[:],
                                         vT_sb[:, 2 * jp:2 * jp + 2, :],
                                         rhs, start=(jp == 0),
                                         stop=(jp == NJP - 1), perf_mode=DR)
                    # ii0 denominator in-loop (1 PSUM bank); ii1's runs
                    # post-loop in an s-pool slot
                    nc.tensor.matmul(ps_m0[:], ones8[:, :, 0:1], rhs0,
                                     start=(jp == 0), stop=(jp == NJP - 1),
                                     perf_mode=DR)
                    es.append(e)
                while wi < len(work):
                    work[wi]()
                    wi += 1

                ps_m1 = ps_s.tile([1, TI], F32, name="ps_m1", tag="s")
                for jp in range(NJP):
                    nc.tensor.matmul(ps_m1[:], ones8[:, :, 0:1],
                                     es[jp][:, 1].rearrange("p t j -> p j t"),
                                     start=(jp == 0), stop=(jp == NJP - 1),
                                     perf_mode=DR)

                attn0 = normalize(0, ps_m0, ps_a0)
                attn1 = normalize(1, ps_m1, ps_a1)
                if last:
                    ffn(xs[0], 0)
                    ffn(xs[1], 1)
                xs[2 * p] = proj(attn0, rpair[0], qpair[0])
                if not last:
                    qpair_n = [qn[0], qn[1]]
                    prime = s_pair(0, qpair_n)
                if last:
                    ffn(xs[2], 2)
                    ffn(xs[3], 3)
                xs[2 * p + 1] = proj(attn1, rpair[1], qpair[1])
                if not last:
                    qpair, rpair = qpair_n, [rn[0], rn[1]]

            # ---- FFN for the last pair ----
            for i in range(2 * (NP - 1), NT):
                ffn(xs[i], i)

    _split_multi_waits(nc)
    return nc


_NC = {}


def _get_nc(with_bias=True):
    if with_bias not in _NC:
        _NC[with_bias] = build_nc(with_bias)
    return _NC[with_bias]


def _prep_core(corr, k, v, w_sk, b_sk, w_proj, b_proj, w_ffn1, b_ffn1,
               w_ffn2, b_ffn2):
    bf = ml_dtypes.bfloat16
    f8 = ml_dtypes.float8_e4m3
    wskT = np.empty((KC, 27, D), dtype=bf)
    for c in range(3):
        for t in range(9):
            dy, dx = t // 3, t % 3
            wskT[:, c * 9 + t, :] = \
                w_sk[:, c * KC:(c + 1) * KC, dy, dx].T.astype(bf)
    vT = v.reshape(D, HW).T.reshape(NJ, 128, D).transpose(1, 0, 2)
    corr_p = np.zeros((CIN, H + 2, W + 2), dtype=bf)
    corr_p[:, 1:H + 1, 1:W + 1] = corr.reshape(CIN, H, W)
    return {
        "corr": corr_p.reshape(CIN, (H + 2) * (W + 2)),
        "k": k.reshape(D, HW).astype(bf),
        "vT": np.ascontiguousarray(vT).astype(f8),
        "wskT": wskT,
        "b_sk": b_sk.reshape(1, D).astype(bf),
        "wprojT": np.ascontiguousarray(
            w_proj.reshape(D, 2 * D).T.reshape(2, D, D)).astype(bf),
        "b_proj": b_proj.reshape(1, D).astype(bf),
        "wf1T": np.ascontiguousarray(w_ffn1.reshape(D, D).T).astype(bf),
        "b_f1": b_ffn1.reshape(D, 1).astype(np.float32),
        "wf2T": np.ascontiguousarray(w_ffn2.reshape(D, D).T).astype(bf),
        "b_f2": b_ffn2.reshape(1, D).astype(bf),
    }


def make_in_maps(corr, k, v, w_sk, b_sk, w_proj, b_proj, w_ffn1, b_ffn1,
                 w_ffn2, b_ffn2):
    corr = np.asarray(corr, dtype=np.float32)
    k = np.asarray(k, dtype=np.float32)
    v = np.asarray(v, dtype=np.float32)
    return [
        _prep_core(corr[i], k[i], v[i], np.asarray(w_sk, np.float32),
                   np.asarray(b_sk, np.float32),
                   np.asarray(w_proj, np.float32),
                   np.asarray(b_proj, np.float32),
                   np.asarray(w_ffn1, np.float32),
                   np.asarray(b_ffn1, np.float32),
                   np.asarray(w_ffn2, np.float32),
                   np.asarray(b_ffn2, np.float32))
        for i in range(N)
    ]


def kernel(corr, k, v, w_sk, b_sk, w_proj, b_proj, w_ffn1, b_ffn1,
           w_ffn2, b_ffn2):
    with_bias = bool(np.any(np.asarray(b_proj)) or np.any(np.asarray(b_ffn2))
                     or np.any(np.asarray(b_sk)))
    nc = _get_nc(with_bias)
    in_maps = make_in_maps(corr, k, v, w_sk, b_sk, w_proj, b_proj,
                           w_ffn1, b_ffn1, w_ffn2, b_ffn2)
    res = run_bass_kernel_spmd(nc, in_maps, list(range(N)))
    out = np.stack([res.results[i]["out"].reshape(D, H, W) for i in range(N)])
    return out.astype(np.float32)
